# revision 25
# baseline (speedup 1.0000x reference)
"""TRN2 Bass kernel for the LSQ-quantized 2-layer MLP.

reference computation:
    wq1 = lsq_quant(w1, alpha1); wq2 = lsq_quant(w2, alpha2)   (tiny 256x256)
    h = relu(x @ wq1.T + b1)
    y = sigmoid(h @ wq2.T + b2)                                 x: [262144, 256] f32

Data-parallel over 8 NeuronCores (32768 tokens/core), no collectives.

Host-side prep per shard (part of sharding):
  * x is transposed to channel-major and cast to f16, so the contraction dim
    lands on SBUF partitions with plain contiguous DMAs (no on-chip
    transposes) at half the HBM read bytes.
  * LSQ quantization is split into integer levels k = round(clip(w/a, -8, 7))
    (exactly representable in f16) and the scale a, folded into the
    activations: h = relu(a1*z + b1), y = sigmoid(a2*z + b2). Weights are
    therefore exact on device; the only precision loss is the f16 rounding
    of x / h and of the staged output (~5e-4 max relative error).

Device pipeline, per 1024-token super-macro (one 512 KiB load / two 256 KiB
stores), all in the transposed channel-major domain:
    HWDGE load xT (f16)                                       [sync queue]
    -> fc1: 4 matmuls f16, w1 chunks stationary, N=512 -> hT PSUM (f32)
    -> relu(a1*z) on DVE (ACT when b1 != 0)            -> f16 SBUF
    -> fc2: 4 matmuls f16, w2 chunks stationary, N=512 -> yT PSUM (f32)
    -> sigmoid(a2*z + b2) on ACT (b2 is per-partition in this layout)
    -> f16 SBUF -> HWDGE store yT                             [sync queue]
Host un-transposes/upcasts yT at gather.

The PE runs a gapless matmul stream at ~100% of theoretical f16 peak
(~111 us/core for 8.6 GFLOP); 16 dummy warmup matmuls trip the HAM clock
gate to 2.4 GHz while the first loads are in flight. Measured ~131 us/core
end to end (vs a ~187 us HBM roofline for the f32-in/f32-out variant).
"""

import numpy as np

import concourse.bass as bass
import concourse.mybir as mybir
import concourse.tile as tile
from concourse import bacc
from concourse.bass import ts
from concourse.bass_utils import run_bass_kernel_spmd

N_CORES = 8
N_TOK = 262144
C = 256
TOK_PER_CORE = N_TOK // N_CORES  # 32768
T_MACRO = 512
N_MACROS = TOK_PER_CORE // T_MACRO  # 64
P = 128

F32 = mybir.dt.float32
F16 = mybir.dt.float16

_program_cache = {}


def _build_program(use_b1: bool, use_b2: bool):
    nc = bacc.Bacc("TRN2", target_bir_lowering=False, debug=False, num_devices=N_CORES)

    xt_d = nc.declare_dram_parameter("xt", [C, TOK_PER_CORE], F16, isOutput=False)
    wk_d = nc.declare_dram_parameter("wk", [P, 2, 2 * C], F16, isOutput=False)
    aa_d = nc.declare_dram_parameter("aa", [P, 2], F32, isOutput=False)
    if use_b1:
        b1s_d = nc.declare_dram_parameter("b1s", [P, 2], F32, isOutput=False)
    if use_b2:
        b2s_d = nc.declare_dram_parameter("b2s", [P, 2], F32, isOutput=False)
    yt_d = nc.declare_dram_parameter("yt", [C, TOK_PER_CORE], F16, isOutput=True)

    # 1024-token super-macros: one 1 MiB load / store per pair of compute macros
    xt_v = xt_d.rearrange("(co ci) (m t) -> m ci co t", ci=P, t=2 * T_MACRO)
    yt_v = yt_d.rearrange("(co ci) (m t) -> m ci co t", ci=P, t=2 * T_MACRO)

    with tile.TileContext(nc) as tc:
        with (
            tc.tile_pool(name="const", bufs=1) as const_pool,
            tc.tile_pool(name="sb_xt", bufs=4) as sb_xt,
            tc.tile_pool(name="sb_ht", bufs=4) as sb_ht,
            tc.tile_pool(name="sb_yt", bufs=4) as sb_yt,
            tc.tile_pool(name="ps_h", bufs=4, space="PSUM") as ps_h,
            tc.tile_pool(name="ps_y", bufs=4, space="PSUM") as ps_y,
        ):
            wk = const_pool.tile([P, 2, 2 * C], F16)
            w1k = wk[:, :, :C]
            w2k = wk[:, :, C:]
            nc.scalar.dma_start(w1k, wk_d[:, :, :C])
            nc.scalar.dma_start(w2k, wk_d[:, :, C:])
            aa = const_pool.tile([P, 2], F32)
            nc.scalar.dma_start(aa[:], aa_d[:])
            a1 = aa[:, 0:1]
            a2 = aa[:, 1:2]
            if use_b1:
                b1s = const_pool.tile([P, 2], F32)
                nc.scalar.dma_start(b1s[:], b1s_d[:])
            if use_b2:
                b2s = const_pool.tile([P, 2], F32)
                nc.scalar.dma_start(b2s[:], b2s_d[:])

            warm = const_pool.tile([P, T_MACRO], F16)
            nc.gpsimd.memset(warm[:], 0.0)
            pwarm = ps_h.tile([P, T_MACRO], F32, tag="pht")
            for _ in range(16):
                nc.tensor.matmul(
                    pwarm[:], warm[:, :P], warm[:], start=True, stop=True
                )

            for m in range(N_MACROS // 2):
                # x is pre-cast to f16 on the host: plain HWDGE load, half the bytes
                xt = sb_xt.tile([P, 2, 2 * T_MACRO], F16, tag="xt")
                if m == 0:
                    q = T_MACRO // 2
                    for qi in range(4):
                        nc.sync.dma_start(
                            xt[:, :, qi * q : (qi + 1) * q],
                            xt_v[m][:, :, qi * q : (qi + 1) * q],
                        )
                else:
                    nc.sync.dma_start(xt[:], xt_v[m])

                yt = sb_yt.tile([P, 2, 2 * T_MACRO], F16, tag="yt")
                for s in range(2):
                    tok = ts(s, T_MACRO)
                    # fc1: hT[j_chunk] = sum_c w1k[:,c,jchunk].T @ xT[:,c,:]
                    ht = sb_ht.tile([P, 2, T_MACRO], F16, tag="ht")
                    for j in range(2):
                        pht = ps_h.tile([P, T_MACRO], F32, tag="pht")
                        if m == 0 and s == 0:
                            q = T_MACRO // 2
                            for th in range(2):
                                for c in range(2):
                                    nc.tensor.matmul(
                                        pht[:, ts(th, q)],
                                        w1k[:, c, ts(j, P)],
                                        xt[:, c, ts(th, q)],
                                        start=(c == 0),
                                        stop=(c == 1),
                                    )
                        else:
                            for c in range(2):
                                nc.tensor.matmul(
                                    pht[:],
                                    w1k[:, c, ts(j, P)],
                                    xt[:, c, tok],
                                    start=(c == 0),
                                    stop=(c == 1),
                                )
                        if use_b1:
                            nc.scalar.activation(
                                ht[:, j, :],
                                pht[:],
                                mybir.ActivationFunctionType.Relu,
                                bias=b1s[:, j : j + 1],
                                scale=a1,
                            )
                        else:
                            # relu(a1*z) on DVE: (z * a1) max 0
                            nc.vector.tensor_scalar(
                                ht[:, j, :],
                                pht[:],
                                a1,
                                0.0,
                                mybir.AluOpType.mult,
                                mybir.AluOpType.max,
                            )

                    # fc2: yT[j_chunk] = sum_c w2k[:,c,jchunk].T @ hT[:,c,:]
                    for j in range(2):
                        pyt = ps_y.tile([P, T_MACRO], F32, tag="pyt")
                        for c in range(2):
                            nc.tensor.matmul(
                                pyt[:],
                                w2k[:, c, ts(j, P)],
                                ht[:, c, :],
                                start=(c == 0),
                                stop=(c == 1),
                            )
                        nc.scalar.activation(
                            yt[:, j, tok],
                            pyt[:],
                            mybir.ActivationFunctionType.Sigmoid,
                            bias=b2s[:, j : j + 1] if use_b2 else 0.0,
                            scale=a2,
                        )
                    nc.sync.dma_start(yt_v[m][:, :, tok], yt[:, :, tok])

    nc.compile()
    return nc


def _quantize_lsq_int(w: np.ndarray, alpha) -> tuple[np.ndarray, np.float32]:
    """Integer LSQ levels k = round(clip(w/a, -8, 7)) and effective scale a,
    replicating the reference forward numerics in np float32."""
    one = np.float32(1.0)
    g = one / np.sqrt(np.float32(w.size * 7))
    alpha = np.float32(alpha)
    a = np.float32(alpha * g) + np.float32(alpha * np.float32(one - g))
    t = np.clip((w / a).astype(np.float32), np.float32(-8.0), np.float32(7.0))
    r = (np.round(t) - t).astype(np.float32)
    q = (t + r).astype(np.float32)  # integer levels in [-8, 7]
    return q, a


def _prepare(x, w1, b1, alpha1, w2, b2, alpha2):
    x = np.asarray(x, dtype=np.float32)
    w1 = np.asarray(w1, dtype=np.float32)
    w2 = np.asarray(w2, dtype=np.float32)
    b1 = np.asarray(b1, dtype=np.float32)
    b2 = np.asarray(b2, dtype=np.float32)

    k1, a1 = _quantize_lsq_int(w1, alpha1)
    k2, a2 = _quantize_lsq_int(w2, alpha2)

    # lhsT layouts: w1k[ci, co, j] = k1[j, co*128+ci]
    w1k = k1.T.reshape(2, P, C).transpose(1, 0, 2)
    w2k = k2.T.reshape(2, P, C).transpose(1, 0, 2)
    wk = np.ascontiguousarray(np.concatenate([w1k, w2k], axis=2)).astype(
        np.float16
    )

    use_b1 = bool(np.any(b1))
    use_b2 = bool(np.any(b2))
    key = (use_b1, use_b2)
    if key not in _program_cache:
        _program_cache[key] = _build_program(use_b1, use_b2)
    nc = _program_cache[key]

    aa_cols = np.ascontiguousarray(
        np.stack([np.full(P, a1, np.float32), np.full(P, a2, np.float32)], axis=1)
    )

    in_maps = []
    for i in range(N_CORES):
        shard = x[i * TOK_PER_CORE : (i + 1) * TOK_PER_CORE]
        m = {
            "xt": shard.T.astype(np.float16, order="C"),
            "wk": wk,
            "aa": aa_cols,
        }
        if use_b1:
            m["b1s"] = np.ascontiguousarray(b1.reshape(2, P).T)
        if use_b2:
            m["b2s"] = np.ascontiguousarray(b2.reshape(2, P).T)
        in_maps.append(m)
    return nc, in_maps


def kernel(x, w1, b1, alpha1, w2, b2, alpha2):
    nc, in_maps = _prepare(x, w1, b1, alpha1, w2, b2, alpha2)
    res = run_bass_kernel_spmd(nc, in_maps, list(range(N_CORES)))
    out = np.concatenate(
        [res.results[i]["yt"].T.astype(np.float32, order="C") for i in range(N_CORES)],
        axis=0,
    )
    return out


# revision 26
# speedup vs baseline: 1.0013x; 1.0013x over previous
"""TRN2 Bass kernel for the LSQ-quantized 2-layer MLP.

reference computation:
    wq1 = lsq_quant(w1, alpha1); wq2 = lsq_quant(w2, alpha2)   (tiny 256x256)
    h = relu(x @ wq1.T + b1)
    y = sigmoid(h @ wq2.T + b2)                                 x: [262144, 256] f32

Data-parallel over 8 NeuronCores (32768 tokens/core), no collectives.

Host-side prep per shard (part of sharding):
  * x is transposed to channel-major and cast to f16, so the contraction dim
    lands on SBUF partitions with plain contiguous DMAs (no on-chip
    transposes) at half the HBM read bytes.
  * LSQ quantization is split into integer levels k = round(clip(w/a, -8, 7))
    (exactly representable in f16) and the scale a, folded into the
    activations: h = relu(a1*z + b1), y = sigmoid(a2*z + b2). Weights are
    therefore exact on device; the only precision loss is the f16 rounding
    of x / h and of the staged output (~5e-4 max relative error).

Device pipeline, per 1024-token super-macro (one 512 KiB load / two 256 KiB
stores), all in the transposed channel-major domain:
    HWDGE load xT (f16)                                       [sync queue]
    -> fc1: 4 matmuls f16, w1 chunks stationary, N=512 -> hT PSUM (f32)
    -> relu(a1*z) on DVE (ACT when b1 != 0)            -> f16 SBUF
    -> fc2: 4 matmuls f16, w2 chunks stationary, N=512 -> yT PSUM (f32)
    -> sigmoid(a2*z + b2) on ACT (b2 is per-partition in this layout)
    -> f16 SBUF -> HWDGE store yT                             [sync queue]
Host un-transposes/upcasts yT at gather.

The PE runs a gapless matmul stream at ~100% of theoretical f16 peak
(~111 us/core for 8.6 GFLOP); 16 dummy warmup matmuls trip the HAM clock
gate to 2.4 GHz while the first loads are in flight. Measured ~131 us/core
end to end (vs a ~187 us HBM roofline for the f32-in/f32-out variant).
"""

import numpy as np

import concourse.mybir as mybir
import concourse.tile as tile
from concourse import bacc
from concourse.bass import ts
from concourse.bass_utils import run_bass_kernel_spmd

N_CORES = 8
N_TOK = 262144
C = 256
TOK_PER_CORE = N_TOK // N_CORES  # 32768
T_MACRO = 512
N_MACROS = TOK_PER_CORE // T_MACRO  # 64
P = 128

F32 = mybir.dt.float32
F16 = mybir.dt.float16

_program_cache = {}


def _build_program(use_b1: bool, use_b2: bool):
    nc = bacc.Bacc("TRN2", target_bir_lowering=False, debug=False, num_devices=N_CORES)

    xt_d = nc.declare_dram_parameter("xt", [C, TOK_PER_CORE], F16, isOutput=False)
    wk_d = nc.declare_dram_parameter("wk", [P, 2, 2 * C], F16, isOutput=False)
    aa_d = nc.declare_dram_parameter("aa", [P, 2], F32, isOutput=False)
    if use_b1:
        b1s_d = nc.declare_dram_parameter("b1s", [P, 2], F32, isOutput=False)
    if use_b2:
        b2s_d = nc.declare_dram_parameter("b2s", [P, 2], F32, isOutput=False)
    yt_d = nc.declare_dram_parameter("yt", [C, TOK_PER_CORE], F16, isOutput=True)

    # 1024-token super-macros: one 1 MiB load / store per pair of compute macros
    xt_v = xt_d.rearrange("(co ci) (m t) -> m ci co t", ci=P, t=2 * T_MACRO)
    yt_v = yt_d.rearrange("(co ci) (m t) -> m ci co t", ci=P, t=2 * T_MACRO)

    with tile.TileContext(nc) as tc:
        with (
            tc.tile_pool(name="const", bufs=1) as const_pool,
            tc.tile_pool(name="sb_xt", bufs=4) as sb_xt,
            tc.tile_pool(name="sb_ht", bufs=4) as sb_ht,
            tc.tile_pool(name="sb_yt", bufs=4) as sb_yt,
            tc.tile_pool(name="ps_h", bufs=4, space="PSUM") as ps_h,
            tc.tile_pool(name="ps_y", bufs=4, space="PSUM") as ps_y,
        ):
            wk = const_pool.tile([P, 2, 2 * C], F16)
            w1k = wk[:, :, :C]
            w2k = wk[:, :, C:]
            nc.scalar.dma_start(w1k, wk_d[:, :, :C])
            nc.scalar.dma_start(w2k, wk_d[:, :, C:])
            aa = const_pool.tile([P, 2], F32)
            nc.scalar.dma_start(aa[:], aa_d[:])
            a1 = aa[:, 0:1]
            a2 = aa[:, 1:2]
            if use_b1:
                b1s = const_pool.tile([P, 2], F32)
                nc.scalar.dma_start(b1s[:], b1s_d[:])
            if use_b2:
                b2s = const_pool.tile([P, 2], F32)
                nc.scalar.dma_start(b2s[:], b2s_d[:])

            warm = const_pool.tile([P, T_MACRO], F16)
            nc.gpsimd.memset(warm[:], 0.0)
            pwarm = ps_h.tile([P, T_MACRO], F32, tag="pht")
            for _ in range(16):
                nc.tensor.matmul(
                    pwarm[:], warm[:, :P], warm[:], start=True, stop=True
                )

            for m in range(N_MACROS // 2):
                # x is pre-cast to f16 on the host: plain HWDGE load, half the bytes
                xt = sb_xt.tile([P, 2, 2 * T_MACRO], F16, tag="xt")
                if m == 0:
                    q = T_MACRO // 2
                    for qi in range(4):
                        nc.sync.dma_start(
                            xt[:, :, qi * q : (qi + 1) * q],
                            xt_v[m][:, :, qi * q : (qi + 1) * q],
                        )
                else:
                    nc.sync.dma_start(xt[:], xt_v[m])

                yt = sb_yt.tile([P, 2, 2 * T_MACRO], F16, tag="yt")
                for s in range(2):
                    tok = ts(s, T_MACRO)
                    # fc1: hT[j_chunk] = sum_c w1k[:,c,jchunk].T @ xT[:,c,:]
                    ht = sb_ht.tile([P, 2, T_MACRO], F16, tag="ht")
                    for j in range(2):
                        pht = ps_h.tile([P, T_MACRO], F32, tag="pht")
                        if m == 0 and s == 0:
                            q = T_MACRO // 2
                            for th in range(2):
                                for c in range(2):
                                    nc.tensor.matmul(
                                        pht[:, ts(th, q)],
                                        w1k[:, c, ts(j, P)],
                                        xt[:, c, ts(th, q)],
                                        start=(c == 0),
                                        stop=(c == 1),
                                    )
                        else:
                            for c in range(2):
                                nc.tensor.matmul(
                                    pht[:],
                                    w1k[:, c, ts(j, P)],
                                    xt[:, c, tok],
                                    start=(c == 0),
                                    stop=(c == 1),
                                )
                        if use_b1:
                            nc.scalar.activation(
                                ht[:, j, :],
                                pht[:],
                                mybir.ActivationFunctionType.Relu,
                                bias=b1s[:, j : j + 1],
                                scale=a1,
                            )
                        else:
                            # relu(a1*z) on DVE: (z * a1) max 0
                            nc.vector.tensor_scalar(
                                ht[:, j, :],
                                pht[:],
                                a1,
                                0.0,
                                mybir.AluOpType.mult,
                                mybir.AluOpType.max,
                            )

                    # fc2: yT[j_chunk] = sum_c w2k[:,c,jchunk].T @ hT[:,c,:]
                    for j in range(2):
                        pyt = ps_y.tile([P, T_MACRO], F32, tag="pyt")
                        for c in range(2):
                            nc.tensor.matmul(
                                pyt[:],
                                w2k[:, c, ts(j, P)],
                                ht[:, c, :],
                                start=(c == 0),
                                stop=(c == 1),
                            )
                        nc.scalar.activation(
                            yt[:, j, tok],
                            pyt[:],
                            mybir.ActivationFunctionType.Sigmoid,
                            bias=b2s[:, j : j + 1] if use_b2 else 0.0,
                            scale=a2,
                        )
                    nc.sync.dma_start(yt_v[m][:, :, tok], yt[:, :, tok])

    nc.compile()
    return nc


def _quantize_lsq_int(w: np.ndarray, alpha) -> tuple[np.ndarray, np.float32]:
    """Integer LSQ levels k = round(clip(w/a, -8, 7)) and effective scale a,
    replicating the reference forward numerics in np float32."""
    one = np.float32(1.0)
    g = one / np.sqrt(np.float32(w.size * 7))
    alpha = np.float32(alpha)
    a = np.float32(alpha * g) + np.float32(alpha * np.float32(one - g))
    t = np.clip((w / a).astype(np.float32), np.float32(-8.0), np.float32(7.0))
    r = (np.round(t) - t).astype(np.float32)
    q = (t + r).astype(np.float32)  # integer levels in [-8, 7]
    return q, a


def _prepare(x, w1, b1, alpha1, w2, b2, alpha2):
    x = np.asarray(x, dtype=np.float32)
    w1 = np.asarray(w1, dtype=np.float32)
    w2 = np.asarray(w2, dtype=np.float32)
    b1 = np.asarray(b1, dtype=np.float32)
    b2 = np.asarray(b2, dtype=np.float32)

    k1, a1 = _quantize_lsq_int(w1, alpha1)
    k2, a2 = _quantize_lsq_int(w2, alpha2)

    # lhsT layouts: w1k[ci, co, j] = k1[j, co*128+ci]
    w1k = k1.T.reshape(2, P, C).transpose(1, 0, 2)
    w2k = k2.T.reshape(2, P, C).transpose(1, 0, 2)
    wk = np.ascontiguousarray(np.concatenate([w1k, w2k], axis=2)).astype(
        np.float16
    )

    use_b1 = bool(np.any(b1))
    use_b2 = bool(np.any(b2))
    key = (use_b1, use_b2)
    if key not in _program_cache:
        _program_cache[key] = _build_program(use_b1, use_b2)
    nc = _program_cache[key]

    aa_cols = np.ascontiguousarray(
        np.stack([np.full(P, a1, np.float32), np.full(P, a2, np.float32)], axis=1)
    )

    in_maps = []
    for i in range(N_CORES):
        shard = x[i * TOK_PER_CORE : (i + 1) * TOK_PER_CORE]
        m = {
            "xt": shard.T.astype(np.float16, order="C"),
            "wk": wk,
            "aa": aa_cols,
        }
        if use_b1:
            m["b1s"] = np.ascontiguousarray(b1.reshape(2, P).T)
        if use_b2:
            m["b2s"] = np.ascontiguousarray(b2.reshape(2, P).T)
        in_maps.append(m)
    return nc, in_maps


def kernel(x, w1, b1, alpha1, w2, b2, alpha2):
    nc, in_maps = _prepare(x, w1, b1, alpha1, w2, b2, alpha2)
    res = run_bass_kernel_spmd(nc, in_maps, list(range(N_CORES)))
    out = np.concatenate(
        [res.results[i]["yt"].T.astype(np.float32, order="C") for i in range(N_CORES)],
        axis=0,
    )
    return out
